# revision 14
# baseline (speedup 1.0000x reference)
"""Causal self-attention on 8 TRN2 NeuronCores.

Sharding: core c -> (batch b = c//2, head-group g = c%2).
B=4, T=2048, D=1024, 16 heads x 64. Each core computes attention for its
batch and its 8 heads, plus the partial output projection for those heads;
the host sums the two partial projections per batch.

v2 design (vs the phase-serial v1):
  * all inputs shipped bf16; weights loaded once as large contiguous tiles
  * heads processed in PAIRS (A on partitions 0-63, B on 64-127); the two
    S^T matmuls of a pair run CONCURRENTLY on the PE via 64x128 row tiling
    (tile_position derives from the operand base partitions)
  * one [128,1024] exp per k-block covers both heads (A|B in adjacent
    PSUM banks)
  * QKV projection of pair p+1 and output-projection chunks are emitted as
    PE filler inside pair p's attention stream, so the PE never idles long
    enough for HAM to re-throttle and the ACT-bound exp stream is hidden
    behind PE work
Device layouts (host pre-transposes; all bf16 except yT):
  xT    [1024, 2048]  x[b].T
  wqkT  [1024, 1024]  cols p*128..  q-feats of head pair p, +512 k-feats
  wvT   [1024, 512]   cols p*128.. v-feats of pair p
  wpT   [512, 1024]   w_proj[:, group].T
  mask  [128, 2048]   4 causal 0/1 patterns of [128,512] for offsets 0..384
Output: yT [1024, 2048] f32 partial y[b].T (sum over this core's heads).
"""

import sys

for _p in ("/opt/pypackages", "/opt/trn_rl_repo"):
    if _p not in sys.path:
        sys.path.insert(0, _p)

from contextlib import ExitStack

import ml_dtypes
import numpy as np

import concourse.bass as bass
import concourse.tile as tile
from concourse import bacc, mybir
from concourse.bass_utils import run_bass_kernel_spmd

F32 = mybir.dt.float32
BF16 = mybir.dt.bfloat16
AF = mybir.ActivationFunctionType
OP = mybir.AluOpType

D = 1024
T = 2048
DH = 64
GF = 512            # features per group (8 heads)
NP = 4              # head pairs per core

LAST_RESULTS = None
_CACHED = None


def build_program():
    nc = bacc.Bacc("TRN2", target_bir_lowering=False, debug=False)

    xT_d = nc.dram_tensor("xT", [D, T], BF16, kind="ExternalInput").ap()
    wqk_d = nc.dram_tensor("wqkT", [D, 2 * GF], BF16, kind="ExternalInput").ap()
    wv_d = nc.dram_tensor("wvT", [D, GF], BF16, kind="ExternalInput").ap()
    wp_d = nc.dram_tensor("wpT", [GF, D], BF16, kind="ExternalInput").ap()
    mask_d = nc.dram_tensor("mask", [128, 2048], BF16, kind="ExternalInput").ap()
    yT_d = nc.dram_tensor("yT", [D, T], F32, kind="ExternalOutput").ap()

    with tile.TileContext(nc) as tc:
        with ExitStack() as octx:
            # ---- persistent pools --------------------------------------
            c_pool = octx.enter_context(tc.tile_pool(name="const", bufs=1))
            x_pool = octx.enter_context(tc.tile_pool(name="xT", bufs=1))
            wqk_pool = octx.enter_context(tc.tile_pool(name="wqk", bufs=1))
            wv_pool = octx.enter_context(tc.tile_pool(name="wv", bufs=1))
            wp_pool = octx.enter_context(tc.tile_pool(name="wp", bufs=1))
            qk_pool = octx.enter_context(tc.tile_pool(name="qkT", bufs=1))
            v_pool = octx.enter_context(tc.tile_pool(name="vN", bufs=1))
            o_pool = octx.enter_context(tc.tile_pool(name="outT", bufs=1))
            pt_pool = octx.enter_context(tc.tile_pool(name="pt", bufs=3))
            r_pool = octx.enter_context(tc.tile_pool(name="recip", bufs=4))
            y_pool = octx.enter_context(tc.tile_pool(name="y", bufs=4))
            ps_s = octx.enter_context(tc.tile_pool(name="ps_s", bufs=2, space="PSUM"))
            ps_av = octx.enter_context(tc.tile_pool(name="ps_av", bufs=1, space="PSUM"))
            ps_f = octx.enter_context(tc.tile_pool(name="ps_f", bufs=2, space="PSUM"))

            mask_t = c_pool.tile([128, 2048], BF16, name="mask", tag="mask")
            nc.sync.dma_start(mask_t[:], mask_d[:])

            x_t = [x_pool.tile([128, T], BF16, name=f"x{k}", tag=f"x{k}") for k in range(8)]
            for k in range(8):
                nc.sync.dma_start(x_t[k][:], xT_d[k * 128:(k + 1) * 128, :])
            wqk_t = [wqk_pool.tile([128, 2 * GF], BF16, name=f"wqk{k}", tag=f"wqk{k}") for k in range(8)]
            for k in range(8):
                nc.sync.dma_start(wqk_t[k][:], wqk_d[k * 128:(k + 1) * 128, :])
            wv_t = [wv_pool.tile([128, GF], BF16, name=f"wv{k}", tag=f"wv{k}") for k in range(8)]
            for k in range(8):
                nc.sync.dma_start(wv_t[k][:], wv_d[k * 128:(k + 1) * 128, :])
            wp_t = [wp_pool.tile([128, D], BF16, name=f"wp{k}", tag=f"wp{k}") for k in range(4)]
            for k in range(4):
                nc.sync.dma_start(wp_t[k][:], wp_d[k * 128:(k + 1) * 128, :])

            # qkT: per pair p, tile 2p = qT, 2p+1 = kT; rows 0-63 head A
            # features, 64-127 head B
            qk_t = [qk_pool.tile([128, T], BF16, name=f"qk{m}", tag=f"qk{m}") for m in range(8)]
            # vN: 16 tiles [128 t, 520]; cols h*65+0..63 v-feats, h*65+64 ones
            v_t = [v_pool.tile([128, 8 * (DH + 1)], BF16, name=f"v{t}", tag=f"v{t}") for t in range(16)]
            for t in range(16):
                dst = v_t[t][:].rearrange("p (h e) -> p h e", h=8, e=65)[:, :, 64:65]
                nc.vector.memset(dst, 1.0)
            # outT: per pair p [128, 2048]; rows 0-63 head A out, 64-127 B
            out_t = [o_pool.tile([128, T], BF16, name=f"o{k}", tag=f"o{k}") for k in range(4)]

            # ---- HAM warmup: PE busy from t=0 (no DMA dependency) -------
            wmc = c_pool.tile([128, 512], BF16, name="wmc", tag="wmc")
            nc.vector.memset(wmc[:], 1.0)
            wm = ps_f.tile([128, 512], F32, name="warm", tag="psf")
            for _ in range(36):
                nc.tensor.matmul(
                    wm[:], (wmc[:, 0:128]), (wmc[:]),
                    start=True, stop=True, skip_group_check=True,
                )

            # ---- filler generators (QKV projection / output projection) --
            def emit_qk_unit(p, which, n):
                # q (which=0) / k (which=1) features of pair p, t-chunk n
                ps = ps_f.tile([128, 512], F32, name="psqk", tag="psf")
                col = which * 512 + p * 128
                for k in range(8):
                    nc.tensor.matmul(
                        ps[:], (wqk_t[k][:, col:col + 128]),
                        (x_t[k][:, n * 512:(n + 1) * 512]),
                        start=(k == 0), stop=(k == 7), skip_group_check=True,
                    )
                nc.vector.tensor_copy(qk_t[2 * p + which][:, n * 512:(n + 1) * 512], ps[:])

            def emit_v_unit(t):
                # v features of ALL heads for t-tile t (natural orientation)
                ps = ps_f.tile([128, 512], F32, name="psv", tag="psf")
                for k in range(8):
                    nc.tensor.matmul(
                        ps[:], (x_t[k][:, t * 128:(t + 1) * 128]),
                        (wv_t[k][:]),
                        start=(k == 0), stop=(k == 7), skip_group_check=True,
                    )
                src = ps[:].rearrange("p (h e) -> p h e", h=8, e=64)
                dst = v_t[t][:].rearrange("p (h e) -> p h e", h=8, e=65)[:, :, 0:64]
                nc.vector.tensor_copy(dst, src)

            def qk_units(p):
                for which in range(2):
                    for n in range(4):
                        yield ('qk', p, which, n)

            def emit_proj_unit(m, n):
                # y[m-feats, t-chunk n] = sum_p wp_p^T @ out_p
                ps = ps_f.tile([128, 512], F32, name="psy", tag="psf")
                for kk in range(4):
                    nc.tensor.matmul(
                        ps[:], (wp_t[kk][:, m * 128:(m + 1) * 128]),
                        (out_t[kk][:, n * 512:(n + 1) * 512]),
                        start=(kk == 0), stop=(kk == 3), skip_group_check=True,
                    )
                yt = y_pool.tile([128, 512], F32, name="yst", tag="yst")
                nc.vector.tensor_copy(yt[:], ps[:])
                nc.sync.dma_start(
                    yT_d[m * 128:(m + 1) * 128, n * 512:(n + 1) * 512], yt[:]
                )

            def emit_unit(u):
                if u[0] == 'qk':
                    emit_qk_unit(u[1], u[2], u[3])
                elif u[0] == 'v':
                    emit_v_unit(u[1])
                else:
                    emit_proj_unit(u[1], u[2])

            # ---- global filler queue with deadlines ---------------------
            # clock = 4*pair + chunk. A unit's deadline is the clock of the
            # first attention chunk that reads its output; proj units have
            # no deadline but only become available once all four pairs'
            # out_t chunk n exists (tracked via p3norm).
            queue = []                   # (deadline, seq, unit)
            seq = 0
            for t in range(16):
                queue.append((t // 4, seq, ('v', t))); seq += 1
            for p in range(NP):
                for which in range(2):
                    for n in range(4):
                        queue.append((4 * p + n, seq, ('qk', p, which, n))); seq += 1
            for n in range(4):
                for m in range(8):
                    queue.append((99, seq, ('proj', m, n))); seq += 1
            queue.sort()
            st = {'p3norm': -1}

            def available(u):
                if u[0] == 'proj':
                    return st['p3norm'] >= u[2]
                return True

            def force(clock):
                # emit every unit the upcoming chunk depends on
                while queue and queue[0][0] <= clock:
                    emit_unit(queue.pop(0)[2])

            def take(k):
                # emit up to k available units, earliest deadline first
                for _ in range(k):
                    for i, (dl, _s, u) in enumerate(queue):
                        if available(u):
                            queue.pop(i)
                            emit_unit(u)
                            break
                    else:
                        return

            # ---- attention over head pairs, QKV/proj interleaved -------
            for p in range(NP):
                qT = qk_t[2 * p]
                kT = qk_t[2 * p + 1]
                sA = 2 * p * 65          # v_t column slots
                sB = (2 * p + 1) * 65

                for c in range(4):
                    force(4 * p + c)
                    npieces = 4 * c + 4
                    avA = ps_av.tile([65, 512], F32, name="avA", tag="avA")
                    avB = ps_av.tile([65, 512], F32, name="avB", tag="avB")
                    for w in range(0, npieces, 2):
                        ss = []
                        for idx in range(2):
                            j = w + idx
                            s = ps_s.tile([128, 1024], F32, name="s", tag="s")
                            nc.tensor.matmul(
                                s[:, 0:512],
                                (kT[0:64, j * 128:(j + 1) * 128]),
                                (qT[0:64, c * 512:(c + 1) * 512]),
                                start=True, stop=True, skip_group_check=True,
                            )
                            nc.tensor.matmul(
                                s[:, 512:1024],
                                (kT[64:128, j * 128:(j + 1) * 128]),
                                (qT[64:128, c * 512:(c + 1) * 512]),
                                start=True, stop=True, skip_group_check=True,
                            )
                            ss.append(s)
                        pts = []
                        for idx in range(2):
                            j = w + idx
                            pt = pt_pool.tile([128, 1024], BF16, name="pt", tag="pt")
                            nc.scalar.activation(pt[:], ss[idx][:], AF.Exp, scale=0.125)
                            if j // 4 == c:  # diagonal block -> causal mask
                                mp = (j * 128 - c * 512) // 128
                                for half in range(2):
                                    nc.vector.tensor_tensor(
                                        pt[:, half * 512:(half + 1) * 512],
                                        pt[:, half * 512:(half + 1) * 512],
                                        mask_t[:, mp * 512:(mp + 1) * 512],
                                        op=OP.mult,
                                    )
                            pts.append(pt)
                        for idx in range(2):
                            j = w + idx
                            nc.tensor.matmul(
                                avA[:], (v_t[j][:, sA:sA + 65]),
                                (pts[idx][:, 0:512]),
                                start=(j == 0), stop=(j == npieces - 1),
                                skip_group_check=True,
                            )
                            nc.tensor.matmul(
                                avB[:], (v_t[j][:, sB:sB + 65]),
                                (pts[idx][:, 512:1024]),
                                start=(j == 0), stop=(j == npieces - 1),
                                skip_group_check=True,
                            )
                        # one filler unit per j-pair keeps the PE queue fed
                        # while ACT works through the exps
                        take(1)
                    # ---- normalize + evacuate this q-chunk ----
                    # evacuate av -> out_t FIRST (frees the PSUM bank for the
                    # next chunk ~2us earlier), then normalize out_t in place
                    for av, row0 in ((avA, 0), (avB, 64)):
                        oslice = out_t[p][row0:row0 + 64, c * 512:(c + 1) * 512]
                        den = r_pool.tile([1, 512], F32, name="den", tag="den")
                        nc.vector.tensor_copy(den[:], av[64:65, :])
                        stg = r_pool.tile([64, 512], F32, name="stg", tag="stg")
                        nc.vector.tensor_copy(stg[:], av[0:64, :])
                        scr = r_pool.tile([1, 512], F32, name="scr", tag="scr")
                        rec = r_pool.tile([1, 512], F32, name="rec", tag="rec")
                        nc.vector.reciprocal_approx_accurate(rec[:], den[:], scratch=scr[:])
                        rb = r_pool.tile([64, 512], F32, name="rb", tag="rb")
                        nc.gpsimd.partition_broadcast(rb[:], rec[:])
                        nc.vector.tensor_tensor(oslice, stg[:], rb[:], op=OP.mult)
                    if p == NP - 1:
                        st['p3norm'] = c
                    # cover the norm chain + next chunk's AV-bank wait with
                    # PE filler work
                    take(2)
            # flush remaining filler (final projection chunk)
            take(len(queue))


    nc.compile()
    return nc


def _make_mask():
    mask = np.zeros((128, 2048), dtype=np.float32)
    kk = np.arange(128)[:, None]
    q = np.arange(512)[None, :]
    for p in range(4):
        d = 128 * p
        mask[:, p * 512:(p + 1) * 512] = ((q - d) >= kk).astype(np.float32)
    return mask


def kernel(x, w_qkv, w_proj):
    global LAST_RESULTS, _CACHED
    x = np.asarray(x, dtype=np.float32)
    w_qkv = np.asarray(w_qkv, dtype=np.float32)
    w_proj = np.asarray(w_proj, dtype=np.float32)
    B = x.shape[0]

    if _CACHED is None:
        _CACHED = build_program()
    nc = _CACHED

    mask = _make_mask()
    in_maps = []
    for c in range(8):
        b, g = c // 2, c % 2
        wq = w_qkv[g * GF:(g + 1) * GF, :]                # [512, 1024]
        wk = w_qkv[D + g * GF: D + (g + 1) * GF, :]
        wv = w_qkv[2 * D + g * GF: 2 * D + (g + 1) * GF, :]
        in_maps.append({
            "xT": np.ascontiguousarray(x[b].T).astype(ml_dtypes.bfloat16),
            "wqkT": np.ascontiguousarray(np.concatenate([wq, wk], axis=0).T).astype(ml_dtypes.bfloat16),
            "wvT": np.ascontiguousarray(wv.T).astype(ml_dtypes.bfloat16),
            "wpT": np.ascontiguousarray(w_proj[:, g * GF:(g + 1) * GF].T).astype(ml_dtypes.bfloat16),
            "mask": mask.astype(ml_dtypes.bfloat16),
        })

    res = run_bass_kernel_spmd(nc, in_maps, core_ids=list(range(8)))
    LAST_RESULTS = res

    y = np.empty_like(x)
    for b in range(B):
        yT = res.results[2 * b]["yT"] + res.results[2 * b + 1]["yT"]
        y[b] = yT.T
    return y


# revision 16
# speedup vs baseline: 1.0184x; 1.0184x over previous
"""Causal self-attention on 8 TRN2 NeuronCores.

Sharding: core c -> (batch b = c//2, head-group g = c%2).
B=4, T=2048, D=1024, 16 heads x 64. Each core computes attention for its
batch and its 8 heads, plus the partial output projection for those heads;
the host sums the two partial projections per batch.

v2 design (vs the phase-serial v1):
  * all inputs shipped bf16; weights loaded once as large contiguous tiles
  * heads processed in PAIRS (A on partitions 0-63, B on 64-127); the two
    S^T matmuls of a pair run CONCURRENTLY on the PE via 64x128 row tiling
    (tile_position derives from the operand base partitions)
  * one [128,1024] exp per k-block covers both heads (A|B in adjacent
    PSUM banks)
  * QKV projection of pair p+1 and output-projection chunks are emitted as
    PE filler inside pair p's attention stream, so the PE never idles long
    enough for HAM to re-throttle and the ACT-bound exp stream is hidden
    behind PE work
Device layouts (host pre-transposes; all bf16 except yT):
  xT    [1024, 2048]  x[b].T
  wqkT  [1024, 1024]  cols p*128..  q-feats of head pair p, +512 k-feats
  wvT   [1024, 512]   cols p*128.. v-feats of pair p
  wpT   [512, 1024]   w_proj[:, group].T
  mask  [128, 2048]   4 causal 0/1 patterns of [128,512] for offsets 0..384
Output: yT [1024, 2048] f32 partial y[b].T (sum over this core's heads).
"""

import sys

for _p in ("/opt/pypackages", "/opt/trn_rl_repo"):
    if _p not in sys.path:
        sys.path.insert(0, _p)

from contextlib import ExitStack

import ml_dtypes
import numpy as np

import concourse.bass as bass
import concourse.tile as tile
from concourse import bacc, mybir
from concourse.bass_utils import run_bass_kernel_spmd

F32 = mybir.dt.float32
BF16 = mybir.dt.bfloat16
AF = mybir.ActivationFunctionType
OP = mybir.AluOpType

D = 1024
T = 2048
DH = 64
GF = 512            # features per group (8 heads)
NP = 4              # head pairs per core

LAST_RESULTS = None
_CACHED = None


def build_program():
    nc = bacc.Bacc("TRN2", target_bir_lowering=False, debug=False)

    xT_d = nc.dram_tensor("xT", [D, T], BF16, kind="ExternalInput").ap()
    wqk_d = nc.dram_tensor("wqkT", [D, 2 * GF], BF16, kind="ExternalInput").ap()
    wv_d = nc.dram_tensor("wvT", [D, GF], BF16, kind="ExternalInput").ap()
    wp_d = nc.dram_tensor("wpT", [GF, D], BF16, kind="ExternalInput").ap()
    mask_d = nc.dram_tensor("mask", [128, 2048], BF16, kind="ExternalInput").ap()
    yT_d = nc.dram_tensor("yT", [D, T], F32, kind="ExternalOutput").ap()

    with tile.TileContext(nc) as tc:
        with ExitStack() as octx:
            # ---- persistent pools --------------------------------------
            c_pool = octx.enter_context(tc.tile_pool(name="const", bufs=1))
            x_pool = octx.enter_context(tc.tile_pool(name="xT", bufs=1))
            wqk_pool = octx.enter_context(tc.tile_pool(name="wqk", bufs=1))
            wv_pool = octx.enter_context(tc.tile_pool(name="wv", bufs=1))
            wp_pool = octx.enter_context(tc.tile_pool(name="wp", bufs=1))
            qk_pool = octx.enter_context(tc.tile_pool(name="qkT", bufs=1))
            v_pool = octx.enter_context(tc.tile_pool(name="vN", bufs=1))
            o_pool = octx.enter_context(tc.tile_pool(name="outT", bufs=1))
            pt_pool = octx.enter_context(tc.tile_pool(name="pt", bufs=3))
            r_pool = octx.enter_context(tc.tile_pool(name="recip", bufs=4))
            y_pool = octx.enter_context(tc.tile_pool(name="y", bufs=4))
            ps_s = octx.enter_context(tc.tile_pool(name="ps_s", bufs=2, space="PSUM"))
            ps_av = octx.enter_context(tc.tile_pool(name="ps_av", bufs=1, space="PSUM"))
            ps_f = octx.enter_context(tc.tile_pool(name="ps_f", bufs=2, space="PSUM"))

            mask_t = c_pool.tile([128, 2048], BF16, name="mask", tag="mask")
            nc.sync.dma_start(mask_t[:], mask_d[:])

            x_t = [x_pool.tile([128, T], BF16, name=f"x{k}", tag=f"x{k}") for k in range(8)]
            for k in range(8):
                nc.sync.dma_start(x_t[k][:], xT_d[k * 128:(k + 1) * 128, :])
            wqk_t = [wqk_pool.tile([128, 2 * GF], BF16, name=f"wqk{k}", tag=f"wqk{k}") for k in range(8)]
            for k in range(8):
                nc.sync.dma_start(wqk_t[k][:], wqk_d[k * 128:(k + 1) * 128, :])
            wv_t = [wv_pool.tile([128, GF], BF16, name=f"wv{k}", tag=f"wv{k}") for k in range(8)]
            for k in range(8):
                nc.sync.dma_start(wv_t[k][:], wv_d[k * 128:(k + 1) * 128, :])
            wp_t = [wp_pool.tile([128, D], BF16, name=f"wp{k}", tag=f"wp{k}") for k in range(4)]
            for k in range(4):
                nc.sync.dma_start(wp_t[k][:], wp_d[k * 128:(k + 1) * 128, :])

            # qkT: per pair p, tile 2p = qT, 2p+1 = kT; rows 0-63 head A
            # features, 64-127 head B
            qk_t = [qk_pool.tile([128, T], BF16, name=f"qk{m}", tag=f"qk{m}") for m in range(8)]
            # vN: 16 tiles [128 t, 520]; cols h*65+0..63 v-feats, h*65+64 ones
            v_t = [v_pool.tile([128, 8 * (DH + 1)], BF16, name=f"v{t}", tag=f"v{t}") for t in range(16)]
            for t in range(16):
                dst = v_t[t][:].rearrange("p (h e) -> p h e", h=8, e=65)[:, :, 64:65]
                nc.vector.memset(dst, 1.0)
            # outT: per pair p [128, 2048]; rows 0-63 head A out, 64-127 B
            out_t = [o_pool.tile([128, T], BF16, name=f"o{k}", tag=f"o{k}") for k in range(4)]

            # ---- HAM warmup: PE busy from t=0 (no DMA dependency) -------
            wmc = c_pool.tile([128, 512], BF16, name="wmc", tag="wmc")
            nc.vector.memset(wmc[:], 1.0)
            wm = ps_f.tile([128, 512], F32, name="warm", tag="psf")
            for _ in range(80):
                nc.tensor.matmul(
                    wm[:], (wmc[:, 0:128]), (wmc[:]),
                    start=True, stop=True, skip_group_check=True,
                )

            # ---- filler generators (QKV projection / output projection) --
            def emit_qk_unit(p, which, n):
                # q (which=0) / k (which=1) features of pair p, t-chunk n
                ps = ps_f.tile([128, 512], F32, name="psqk", tag="psf")
                col = which * 512 + p * 128
                for k in range(8):
                    nc.tensor.matmul(
                        ps[:], (wqk_t[k][:, col:col + 128]),
                        (x_t[k][:, n * 512:(n + 1) * 512]),
                        start=(k == 0), stop=(k == 7), skip_group_check=True,
                    )
                nc.vector.tensor_copy(qk_t[2 * p + which][:, n * 512:(n + 1) * 512], ps[:])

            def emit_v_unit(t):
                # v features of ALL heads for t-tile t (natural orientation)
                ps = ps_f.tile([128, 512], F32, name="psv", tag="psf")
                for k in range(8):
                    nc.tensor.matmul(
                        ps[:], (x_t[k][:, t * 128:(t + 1) * 128]),
                        (wv_t[k][:]),
                        start=(k == 0), stop=(k == 7), skip_group_check=True,
                    )
                src = ps[:].rearrange("p (h e) -> p h e", h=8, e=64)
                dst = v_t[t][:].rearrange("p (h e) -> p h e", h=8, e=65)[:, :, 0:64]
                nc.vector.tensor_copy(dst, src)

            def qk_units(p):
                for which in range(2):
                    for n in range(4):
                        yield ('qk', p, which, n)

            def emit_proj_unit(m, n):
                # y[m-feats, t-chunk n] = sum_p wp_p^T @ out_p
                ps = ps_f.tile([128, 512], F32, name="psy", tag="psf")
                for kk in range(4):
                    nc.tensor.matmul(
                        ps[:], (wp_t[kk][:, m * 128:(m + 1) * 128]),
                        (out_t[kk][:, n * 512:(n + 1) * 512]),
                        start=(kk == 0), stop=(kk == 3), skip_group_check=True,
                    )
                yt = y_pool.tile([128, 512], F32, name="yst", tag="yst")
                nc.vector.tensor_copy(yt[:], ps[:])
                nc.sync.dma_start(
                    yT_d[m * 128:(m + 1) * 128, n * 512:(n + 1) * 512], yt[:]
                )

            def emit_unit(u):
                if u[0] == 'qk':
                    emit_qk_unit(u[1], u[2], u[3])
                elif u[0] == 'v':
                    emit_v_unit(u[1])
                else:
                    emit_proj_unit(u[1], u[2])

            # ---- wavefront block order + deadline filler queue ---------
            # Attention (pair, chunk) blocks run in anti-diagonal order:
            # this spreads the exp-heavy late chunks (ACT work ~ c+1) evenly
            # across the kernel so QKV/projection filler can keep the PE
            # busy everywhere. A unit's deadline is the position of the
            # first block that reads its output; proj units have no deadline
            # but become available once all four pairs' out_t chunk n exist.
            blocks = sorted(
                [(p, c) for p in range(NP) for c in range(4)],
                key=lambda pc: (pc[0] + pc[1], pc[0]),
            )
            bpos = {pc: i for i, pc in enumerate(blocks)}
            queue = []                   # (deadline, seq, unit)
            seq = 0
            for t in range(16):
                queue.append((bpos[(0, t // 4)], seq, ('v', t))); seq += 1
            for p in range(NP):
                for which in range(2):
                    for n in range(4):
                        queue.append((bpos[(p, n)], seq, ('qk', p, which, n))); seq += 1
            for n in range(4):
                for m in range(8):
                    queue.append((99, seq, ('proj', m, n))); seq += 1
            queue.sort()
            st = {'p3norm': -1}

            def available(u):
                if u[0] == 'proj':
                    return st['p3norm'] >= u[2]
                return True

            def force(clock):
                # emit every unit the upcoming block depends on
                while queue and queue[0][0] <= clock:
                    emit_unit(queue.pop(0)[2])

            def take(k):
                # emit up to k available units, earliest deadline first
                for _ in range(k):
                    for i, (dl, _s, u) in enumerate(queue):
                        if available(u):
                            queue.pop(i)
                            emit_unit(u)
                            break
                    else:
                        return

            # ---- attention blocks, QKV/proj interleaved ----------------
            for bi, (p, c) in enumerate(blocks):
                qT = qk_t[2 * p]
                kT = qk_t[2 * p + 1]
                sA = 2 * p * 65          # v_t column slots
                sB = (2 * p + 1) * 65
                if True:
                    force(bi)
                    npieces = 4 * c + 4
                    avA = ps_av.tile([65, 512], F32, name="avA", tag="avA")
                    avB = ps_av.tile([65, 512], F32, name="avB", tag="avB")
                    for w in range(0, npieces, 2):
                        ss = []
                        for idx in range(2):
                            j = w + idx
                            s = ps_s.tile([128, 1024], F32, name="s", tag="s")
                            nc.tensor.matmul(
                                s[:, 0:512],
                                (kT[0:64, j * 128:(j + 1) * 128]),
                                (qT[0:64, c * 512:(c + 1) * 512]),
                                start=True, stop=True, skip_group_check=True,
                            )
                            nc.tensor.matmul(
                                s[:, 512:1024],
                                (kT[64:128, j * 128:(j + 1) * 128]),
                                (qT[64:128, c * 512:(c + 1) * 512]),
                                start=True, stop=True, skip_group_check=True,
                            )
                            ss.append(s)
                        pts = []
                        for idx in range(2):
                            j = w + idx
                            pt = pt_pool.tile([128, 1024], BF16, name="pt", tag="pt")
                            nc.scalar.activation(pt[:], ss[idx][:], AF.Exp, scale=0.125)
                            if j // 4 == c:  # diagonal block -> causal mask
                                mp = (j * 128 - c * 512) // 128
                                for half in range(2):
                                    nc.vector.tensor_tensor(
                                        pt[:, half * 512:(half + 1) * 512],
                                        pt[:, half * 512:(half + 1) * 512],
                                        mask_t[:, mp * 512:(mp + 1) * 512],
                                        op=OP.mult,
                                    )
                            pts.append(pt)
                        for idx in range(2):
                            j = w + idx
                            nc.tensor.matmul(
                                avA[:], (v_t[j][:, sA:sA + 65]),
                                (pts[idx][:, 0:512]),
                                start=(j == 0), stop=(j == npieces - 1),
                                skip_group_check=True,
                            )
                            nc.tensor.matmul(
                                avB[:], (v_t[j][:, sB:sB + 65]),
                                (pts[idx][:, 512:1024]),
                                start=(j == 0), stop=(j == npieces - 1),
                                skip_group_check=True,
                            )
                        # one filler unit per j-pair keeps the PE queue fed
                        # while ACT works through the exps
                        take(1)
                    # ---- normalize + evacuate this q-chunk ----
                    # evacuate av -> out_t FIRST (frees the PSUM bank for the
                    # next chunk ~2us earlier), then normalize out_t in place
                    for av, row0 in ((avA, 0), (avB, 64)):
                        oslice = out_t[p][row0:row0 + 64, c * 512:(c + 1) * 512]
                        den = r_pool.tile([1, 512], F32, name="den", tag="den")
                        nc.vector.tensor_copy(den[:], av[64:65, :])
                        stg = r_pool.tile([64, 512], F32, name="stg", tag="stg")
                        nc.vector.tensor_copy(stg[:], av[0:64, :])
                        scr = r_pool.tile([1, 512], F32, name="scr", tag="scr")
                        rec = r_pool.tile([1, 512], F32, name="rec", tag="rec")
                        nc.vector.reciprocal_approx_accurate(rec[:], den[:], scratch=scr[:])
                        rb = r_pool.tile([64, 512], F32, name="rb", tag="rb")
                        nc.gpsimd.partition_broadcast(rb[:], rec[:])
                        nc.vector.tensor_tensor(oslice, stg[:], rb[:], op=OP.mult)
                    if p == NP - 1:
                        st['p3norm'] = c
                    # cover the norm chain + next chunk's AV-bank wait with
                    # PE filler work
                    take(2)
            # flush remaining filler (final projection chunk)
            take(len(queue))


    nc.compile()
    return nc


def _make_mask():
    mask = np.zeros((128, 2048), dtype=np.float32)
    kk = np.arange(128)[:, None]
    q = np.arange(512)[None, :]
    for p in range(4):
        d = 128 * p
        mask[:, p * 512:(p + 1) * 512] = ((q - d) >= kk).astype(np.float32)
    return mask


def kernel(x, w_qkv, w_proj):
    global LAST_RESULTS, _CACHED
    x = np.asarray(x, dtype=np.float32)
    w_qkv = np.asarray(w_qkv, dtype=np.float32)
    w_proj = np.asarray(w_proj, dtype=np.float32)
    B = x.shape[0]

    if _CACHED is None:
        _CACHED = build_program()
    nc = _CACHED

    mask = _make_mask()
    in_maps = []
    for c in range(8):
        b, g = c // 2, c % 2
        wq = w_qkv[g * GF:(g + 1) * GF, :]                # [512, 1024]
        wk = w_qkv[D + g * GF: D + (g + 1) * GF, :]
        wv = w_qkv[2 * D + g * GF: 2 * D + (g + 1) * GF, :]
        in_maps.append({
            "xT": np.ascontiguousarray(x[b].T).astype(ml_dtypes.bfloat16),
            "wqkT": np.ascontiguousarray(np.concatenate([wq, wk], axis=0).T).astype(ml_dtypes.bfloat16),
            "wvT": np.ascontiguousarray(wv.T).astype(ml_dtypes.bfloat16),
            "wpT": np.ascontiguousarray(w_proj[:, g * GF:(g + 1) * GF].T).astype(ml_dtypes.bfloat16),
            "mask": mask.astype(ml_dtypes.bfloat16),
        })

    res = run_bass_kernel_spmd(nc, in_maps, core_ids=list(range(8)))
    LAST_RESULTS = res

    y = np.empty_like(x)
    for b in range(B):
        yT = res.results[2 * b]["yT"] + res.results[2 * b + 1]["yT"]
        y[b] = yT.T
    return y


# revision 18
# speedup vs baseline: 1.0602x; 1.0410x over previous
"""Causal self-attention on 8 TRN2 NeuronCores.

Sharding: core c -> (batch b = c//2, head-group g = c%2).
B=4, T=2048, D=1024, 16 heads x 64. Each core computes attention for its
batch and its 8 heads, plus the partial output projection for those heads;
the host sums the two partial projections per batch.

v2 design (vs the phase-serial v1):
  * all inputs shipped bf16; weights loaded once as large contiguous tiles
  * heads processed in PAIRS (A on partitions 0-63, B on 64-127); the two
    S^T matmuls of a pair run CONCURRENTLY on the PE via 64x128 row tiling
    (tile_position derives from the operand base partitions)
  * one [128,1024] exp per k-block covers both heads (A|B in adjacent
    PSUM banks)
  * QKV projection of pair p+1 and output-projection chunks are emitted as
    PE filler inside pair p's attention stream, so the PE never idles long
    enough for HAM to re-throttle and the ACT-bound exp stream is hidden
    behind PE work
Device layouts (host pre-transposes; all bf16 except yT):
  xT    [1024, 2048]  x[b].T
  wqkT  [1024, 1024]  cols p*128..  q-feats of head pair p, +512 k-feats
  wvT   [1024, 512]   cols p*128.. v-feats of pair p
  wpT   [512, 1024]   w_proj[:, group].T
  mask  [128, 2048]   4 causal 0/1 patterns of [128,512] for offsets 0..384
Output: yT [1024, 2048] f32 partial y[b].T (sum over this core's heads).
"""

import sys

for _p in ("/opt/pypackages", "/opt/trn_rl_repo"):
    if _p not in sys.path:
        sys.path.insert(0, _p)

from contextlib import ExitStack

import ml_dtypes
import numpy as np

import concourse.bass as bass
import concourse.tile as tile
from concourse import bacc, mybir
from concourse.bass_utils import run_bass_kernel_spmd

F32 = mybir.dt.float32
BF16 = mybir.dt.bfloat16
AF = mybir.ActivationFunctionType
OP = mybir.AluOpType

D = 1024
T = 2048
DH = 64
GF = 512            # features per group (8 heads)
NP = 4              # head pairs per core

LAST_RESULTS = None
_CACHED = None


def build_program():
    nc = bacc.Bacc("TRN2", target_bir_lowering=False, debug=False)

    xT_d = nc.dram_tensor("xT", [D, T], BF16, kind="ExternalInput").ap()
    wqk_d = nc.dram_tensor("wqkT", [D, 2 * GF], BF16, kind="ExternalInput").ap()
    wv_d = nc.dram_tensor("wvT", [D, GF], BF16, kind="ExternalInput").ap()
    wp_d = nc.dram_tensor("wpT", [GF, D], BF16, kind="ExternalInput").ap()
    mask_d = nc.dram_tensor("mask", [128, 2048], BF16, kind="ExternalInput").ap()
    yT_d = nc.dram_tensor("yT", [D, T], F32, kind="ExternalOutput").ap()

    with tile.TileContext(nc) as tc:
        with ExitStack() as octx:
            # ---- persistent pools --------------------------------------
            c_pool = octx.enter_context(tc.tile_pool(name="const", bufs=1))
            x_pool = octx.enter_context(tc.tile_pool(name="xT", bufs=1))
            wqk_pool = octx.enter_context(tc.tile_pool(name="wqk", bufs=1))
            wv_pool = octx.enter_context(tc.tile_pool(name="wv", bufs=1))
            wp_pool = octx.enter_context(tc.tile_pool(name="wp", bufs=1))
            qk_pool = octx.enter_context(tc.tile_pool(name="qkT", bufs=1))
            v_pool = octx.enter_context(tc.tile_pool(name="vN", bufs=1))
            o_pool = octx.enter_context(tc.tile_pool(name="outT", bufs=1))
            pt_pool = octx.enter_context(tc.tile_pool(name="pt", bufs=4))
            r_pool = octx.enter_context(tc.tile_pool(name="recip", bufs=4))
            y_pool = octx.enter_context(tc.tile_pool(name="y", bufs=4))
            ps_s = octx.enter_context(tc.tile_pool(name="ps_s", bufs=2, space="PSUM"))
            ps_av = octx.enter_context(tc.tile_pool(name="ps_av", bufs=1, space="PSUM"))
            ps_f = octx.enter_context(tc.tile_pool(name="ps_f", bufs=2, space="PSUM"))

            mask_t = c_pool.tile([128, 2048], BF16, name="mask", tag="mask")
            nc.sync.dma_start(mask_t[:], mask_d[:])

            x_t = [x_pool.tile([128, T], BF16, name=f"x{k}", tag=f"x{k}") for k in range(8)]
            for k in range(8):
                nc.sync.dma_start(x_t[k][:], xT_d[k * 128:(k + 1) * 128, :])
            wqk_t = [wqk_pool.tile([128, 2 * GF], BF16, name=f"wqk{k}", tag=f"wqk{k}") for k in range(8)]
            for k in range(8):
                nc.sync.dma_start(wqk_t[k][:], wqk_d[k * 128:(k + 1) * 128, :])
            wv_t = [wv_pool.tile([128, GF], BF16, name=f"wv{k}", tag=f"wv{k}") for k in range(8)]
            for k in range(8):
                nc.sync.dma_start(wv_t[k][:], wv_d[k * 128:(k + 1) * 128, :])
            wp_t = [wp_pool.tile([128, D], BF16, name=f"wp{k}", tag=f"wp{k}") for k in range(4)]
            for k in range(4):
                nc.sync.dma_start(wp_t[k][:], wp_d[k * 128:(k + 1) * 128, :])

            # qkT: per pair p, tile 2p = qT, 2p+1 = kT; rows 0-63 head A
            # features, 64-127 head B
            qk_t = [qk_pool.tile([128, T], BF16, name=f"qk{m}", tag=f"qk{m}") for m in range(8)]
            # vN: 16 tiles [128 t, 520]; cols h*65+0..63 v-feats, h*65+64 ones
            v_t = [v_pool.tile([128, 8 * (DH + 1)], BF16, name=f"v{t}", tag=f"v{t}") for t in range(16)]
            for t in range(16):
                dst = v_t[t][:].rearrange("p (h e) -> p h e", h=8, e=65)[:, :, 64:65]
                nc.vector.memset(dst, 1.0)
            # outT: per pair p [128, 2048]; rows 0-63 head A out, 64-127 B
            out_t = [o_pool.tile([128, T], BF16, name=f"o{k}", tag=f"o{k}") for k in range(4)]

            # ---- HAM warmup: PE busy from t=0 (no DMA dependency) -------
            wmc = c_pool.tile([128, 512], BF16, name="wmc", tag="wmc")
            nc.vector.memset(wmc[:], 1.0)
            wm = ps_f.tile([128, 512], F32, name="warm", tag="psf")
            for _ in range(80):
                nc.tensor.matmul(
                    wm[:], (wmc[:, 0:128]), (wmc[:]),
                    start=True, stop=True, skip_group_check=True,
                )

            # ---- filler generators (QKV projection / output projection) --
            def emit_qk_unit(p, which, n):
                # q (which=0) / k (which=1) features of pair p, t-chunk n
                ps = ps_f.tile([128, 512], F32, name="psqk", tag="psf")
                col = which * 512 + p * 128
                for k in range(8):
                    nc.tensor.matmul(
                        ps[:], (wqk_t[k][:, col:col + 128]),
                        (x_t[k][:, n * 512:(n + 1) * 512]),
                        start=(k == 0), stop=(k == 7), skip_group_check=True,
                    )
                nc.vector.tensor_copy(qk_t[2 * p + which][:, n * 512:(n + 1) * 512], ps[:])

            def emit_v_unit(t):
                # v features of ALL heads for t-tile t (natural orientation)
                ps = ps_f.tile([128, 512], F32, name="psv", tag="psf")
                for k in range(8):
                    nc.tensor.matmul(
                        ps[:], (x_t[k][:, t * 128:(t + 1) * 128]),
                        (wv_t[k][:]),
                        start=(k == 0), stop=(k == 7), skip_group_check=True,
                    )
                src = ps[:].rearrange("p (h e) -> p h e", h=8, e=64)
                dst = v_t[t][:].rearrange("p (h e) -> p h e", h=8, e=65)[:, :, 0:64]
                nc.vector.tensor_copy(dst, src)

            def qk_units(p):
                for which in range(2):
                    for n in range(4):
                        yield ('qk', p, which, n)

            def emit_proj_unit(m, n):
                # y[m-feats, t-chunk n] = sum_p wp_p^T @ out_p
                ps = ps_f.tile([128, 512], F32, name="psy", tag="psf")
                for kk in range(4):
                    nc.tensor.matmul(
                        ps[:], (wp_t[kk][:, m * 128:(m + 1) * 128]),
                        (out_t[kk][:, n * 512:(n + 1) * 512]),
                        start=(kk == 0), stop=(kk == 3), skip_group_check=True,
                    )
                yt = y_pool.tile([128, 512], F32, name="yst", tag="yst")
                nc.vector.tensor_copy(yt[:], ps[:])
                nc.sync.dma_start(
                    yT_d[m * 128:(m + 1) * 128, n * 512:(n + 1) * 512], yt[:]
                )

            def emit_unit(u):
                if u[0] == 'qk':
                    emit_qk_unit(u[1], u[2], u[3])
                elif u[0] == 'v':
                    emit_v_unit(u[1])
                else:
                    emit_proj_unit(u[1], u[2])

            # ---- wavefront block order + deadline filler queue ---------
            # Attention (pair, chunk) blocks run in anti-diagonal order:
            # this spreads the exp-heavy late chunks (ACT work ~ c+1) evenly
            # across the kernel so QKV/projection filler can keep the PE
            # busy everywhere. A unit's deadline is the position of the
            # first block that reads its output; proj units have no deadline
            # but become available once all four pairs' out_t chunk n exist.
            blocks = sorted(
                [(p, c) for p in range(NP) for c in range(4)],
                key=lambda pc: (pc[0] + pc[1], pc[0]),
            )
            bpos = {pc: i for i, pc in enumerate(blocks)}
            queue = []                   # (deadline, seq, unit)
            seq = 0
            for t in range(16):
                queue.append((bpos[(0, t // 4)], seq, ('v', t))); seq += 1
            for p in range(NP):
                for which in range(2):
                    for n in range(4):
                        queue.append((bpos[(p, n)], seq, ('qk', p, which, n))); seq += 1
            for n in range(4):
                for m in range(8):
                    queue.append((99, seq, ('proj', m, n))); seq += 1
            queue.sort()
            st = {'p3norm': -1}

            def available(u):
                if u[0] == 'proj':
                    return st['p3norm'] >= u[2]
                return True

            def force(clock):
                # emit every unit the upcoming block depends on
                while queue and queue[0][0] <= clock:
                    emit_unit(queue.pop(0)[2])

            def take(k):
                # emit up to k available units, earliest deadline first
                for _ in range(k):
                    for i, (dl, _s, u) in enumerate(queue):
                        if available(u):
                            queue.pop(i)
                            emit_unit(u)
                            break
                    else:
                        return

            # ---- attention blocks, QKV/proj interleaved ----------------
            for bi, (p, c) in enumerate(blocks):
                qT = qk_t[2 * p]
                kT = qk_t[2 * p + 1]
                sA = 2 * p * 65          # v_t column slots
                sB = (2 * p + 1) * 65
                force(bi)
                npieces = 4 * c + 4
                avA = ps_av.tile([65, 512], F32, name="avA", tag="avA")
                avB = ps_av.tile([65, 512], F32, name="avB", tag="avB")

                def emit_S(w):
                    ss = []
                    for idx in range(2):
                        j = w + idx
                        s = ps_s.tile([128, 1024], F32, name="s", tag="s")
                        nc.tensor.matmul(
                            s[:, 0:512],
                            (kT[0:64, j * 128:(j + 1) * 128]),
                            (qT[0:64, c * 512:(c + 1) * 512]),
                            start=True, stop=True, skip_group_check=True,
                        )
                        nc.tensor.matmul(
                            s[:, 512:1024],
                            (kT[64:128, j * 128:(j + 1) * 128]),
                            (qT[64:128, c * 512:(c + 1) * 512]),
                            start=True, stop=True, skip_group_check=True,
                        )
                        ss.append(s)
                    return ss

                def emit_exp(w, ss):
                    pts = []
                    for idx in range(2):
                        j = w + idx
                        pt = pt_pool.tile([128, 1024], BF16, name="pt", tag="pt")
                        nc.scalar.activation(pt[:], ss[idx][:], AF.Exp, scale=0.125)
                        if j // 4 == c:  # diagonal block -> causal mask
                            mp = (j * 128 - c * 512) // 128
                            for half in range(2):
                                nc.vector.tensor_tensor(
                                    pt[:, half * 512:(half + 1) * 512],
                                    pt[:, half * 512:(half + 1) * 512],
                                    mask_t[:, mp * 512:(mp + 1) * 512],
                                    op=OP.mult,
                                )
                        pts.append(pt)
                    return pts

                def emit_pv(w, pts):
                    for idx in range(2):
                        j = w + idx
                        nc.tensor.matmul(
                            avA[:], (v_t[j][:, sA:sA + 65]),
                            (pts[idx][:, 0:512]),
                            start=(j == 0), stop=(j == npieces - 1),
                            skip_group_check=True,
                        )
                        nc.tensor.matmul(
                            avB[:], (v_t[j][:, sB:sB + 65]),
                            (pts[idx][:, 512:1024]),
                            start=(j == 0), stop=(j == npieces - 1),
                            skip_group_check=True,
                        )

                # software pipeline: exp(w) is issued while the PE runs
                # S(w+1) and filler, then PV(w) consumes pt(w) at the tail
                # of the period -- no S->exp->PV latency chain on the
                # in-order PE queue.
                ss = emit_S(0)
                for w in range(0, npieces, 2):
                    pts = emit_exp(w, ss)
                    if w + 2 < npieces:
                        ss = emit_S(w + 2)
                    take(1)
                    emit_pv(w, pts)
                # ---- normalize + evacuate this q-chunk ----
                # evacuate av -> out_t FIRST (frees the PSUM bank for the
                # next chunk ~2us earlier), then normalize out_t in place
                for av, row0 in ((avA, 0), (avB, 64)):
                    oslice = out_t[p][row0:row0 + 64, c * 512:(c + 1) * 512]
                    den = r_pool.tile([1, 512], F32, name="den", tag="den")
                    nc.vector.tensor_copy(den[:], av[64:65, :])
                    stg = r_pool.tile([64, 512], F32, name="stg", tag="stg")
                    nc.vector.tensor_copy(stg[:], av[0:64, :])
                    scr = r_pool.tile([1, 512], F32, name="scr", tag="scr")
                    rec = r_pool.tile([1, 512], F32, name="rec", tag="rec")
                    nc.vector.reciprocal_approx_accurate(rec[:], den[:], scratch=scr[:])
                    rb = r_pool.tile([64, 512], F32, name="rb", tag="rb")
                    nc.gpsimd.partition_broadcast(rb[:], rec[:])
                    nc.vector.tensor_tensor(oslice, stg[:], rb[:], op=OP.mult)
                if p == NP - 1:
                    st['p3norm'] = c
                # cover the norm chain + next chunk's AV-bank wait with
                # PE filler work
                take(2)
            # flush remaining filler (final projection chunk)
            take(len(queue))


    nc.compile()
    return nc


def _make_mask():
    mask = np.zeros((128, 2048), dtype=np.float32)
    kk = np.arange(128)[:, None]
    q = np.arange(512)[None, :]
    for p in range(4):
        d = 128 * p
        mask[:, p * 512:(p + 1) * 512] = ((q - d) >= kk).astype(np.float32)
    return mask


def kernel(x, w_qkv, w_proj):
    global LAST_RESULTS, _CACHED
    x = np.asarray(x, dtype=np.float32)
    w_qkv = np.asarray(w_qkv, dtype=np.float32)
    w_proj = np.asarray(w_proj, dtype=np.float32)
    B = x.shape[0]

    if _CACHED is None:
        _CACHED = build_program()
    nc = _CACHED

    mask = _make_mask()
    in_maps = []
    for c in range(8):
        b, g = c // 2, c % 2
        wq = w_qkv[g * GF:(g + 1) * GF, :]                # [512, 1024]
        wk = w_qkv[D + g * GF: D + (g + 1) * GF, :]
        wv = w_qkv[2 * D + g * GF: 2 * D + (g + 1) * GF, :]
        in_maps.append({
            "xT": np.ascontiguousarray(x[b].T).astype(ml_dtypes.bfloat16),
            "wqkT": np.ascontiguousarray(np.concatenate([wq, wk], axis=0).T).astype(ml_dtypes.bfloat16),
            "wvT": np.ascontiguousarray(wv.T).astype(ml_dtypes.bfloat16),
            "wpT": np.ascontiguousarray(w_proj[:, g * GF:(g + 1) * GF].T).astype(ml_dtypes.bfloat16),
            "mask": mask.astype(ml_dtypes.bfloat16),
        })

    res = run_bass_kernel_spmd(nc, in_maps, core_ids=list(range(8)))
    LAST_RESULTS = res

    y = np.empty_like(x)
    for b in range(B):
        yT = res.results[2 * b]["yT"] + res.results[2 * b + 1]["yT"]
        y[b] = yT.T
    return y
